# revision 7
# baseline (speedup 1.0000x reference)
"""Collision-regularizer loss kernel for 8x TRN2 NeuronCores (Bass/Tile).

Problem: xyz (2, 8192, 3) fp32 -> scalar loss
    loss = mean_{b,i,j} [i!=j] * relu(R - ||p_i - p_j||)^2,  R = 0.1

Algorithm per core (SPMD, identical program, different data):
  - Augmented Gram matmul (K=5): u_ij = |p_i|^2 + |p_j|^2 - 2 p_i.p_j
    computed by PE directly into PSUM, 128x512 per matmul.
  - Let w = min(max(u, 0), R^2). Then relu(R - sqrt(u))^2 = R^2 + w - 2R*sqrt(w).
    So per block we only need Sum(w) (fused accum on the DVE clamp op) and
    Sum(sqrt(w)) (fused accum on the ACT sqrt op). No separate reduction pass.
  - Only strictly-upper blocks of the NxN matrix are computed (host doubles
    the sum); the 16 diagonal-strip blocks use an additive mask input that
    raises killed entries (j <= i) to exactly R^2 => zero loss contribution.
  - Y_ACT blocks instead take a 3-op ACT-only path (sqrt, relu(R-d),
    square+accum) to balance ACT vs DVE load.

Sharding: core k handles batch k//4, row-blocks R = (k%4) + 4t for t=0..15.
Then R//4 == t, so every core runs the identical triangular schedule
(t, c) for c in t..15; the diagonal-mask offset (k%4)*128 is input data.
"""

import sys

import numpy as np

if "/opt/trn_rl_repo" not in sys.path:
    sys.path.insert(0, "/opt/trn_rl_repo")

from contextlib import ExitStack

import concourse.bacc as bacc
import concourse.mybir as mybir
import concourse.tile as tile
from concourse.bass_utils import run_bass_kernel_spmd

F32 = mybir.dt.float32
ALU = mybir.AluOpType
AF = mybir.ActivationFunctionType

B, N, D = 2, 8192, 3
NCORES = 8
NT = 16  # row-blocks (128 rows) per core
NCB = 16  # col-blocks (512 cols) per batch row
R_F32 = np.float32(0.1)
CAP = float(np.float32(R_F32 * R_F32))  # fp32(0.1)^2, the clamp cap
EPS_B = 4e-6  # sqrt guard on the ACT-only path (PE rounding can make u<0)
Y_ACT = 12  # blocks routed to the ACT-only path (load balance knob)

_CACHE = {}


def _schedule(y_act=Y_ACT):
    """Block lists and group structure, identical for all cores."""
    blocks = [(t, c) for t in range(NT) for c in range(t, NCB)]  # 136
    diag = [b for b in blocks if b[0] == b[1]]  # 16
    nondiag = [b for b in blocks if b[0] != b[1]]  # 120
    # ACT-only blocks: the ones farthest from the diagonal
    nd_sorted = sorted(nondiag, key=lambda b: (b[1] - b[0]), reverse=True)
    b_set = set(nd_sorted[:y_act])
    a_nondiag = [b for b in nondiag if b not in b_set]
    a_blocks = a_nondiag + diag  # diag groups last (mask DMA overlap)
    b_blocks = [b for b in nd_sorted[:y_act]]
    a_groups = [a_blocks[i : i + 4] for i in range(0, len(a_blocks), 4)]
    b_groups = [b_blocks[i : i + 4] for i in range(0, len(b_blocks), 4)]
    n_diag_groups = (len(diag) + 3) // 4
    # interleave B-groups evenly among the non-diag A-groups
    order = []  # list of ("A"|"B", group_index)
    na, nb = len(a_groups), len(b_groups)
    stride = max(1, (na - n_diag_groups) // (nb + 1)) if nb else na
    bi = 0
    for ai in range(na):
        order.append(("A", ai))
        if bi < nb and (ai + 1) % stride == 0 and ai < na - n_diag_groups:
            order.append(("B", bi))
            bi += 1
    while bi < nb:
        order.append(("B", bi))
        bi += 1
    n_wtiles = (len(a_groups) + 3) // 4
    return {
        "a_groups": a_groups,
        "b_groups": b_groups,
        "order": order,
        "n_wtiles": n_wtiles,
        "n_a_entries": len(a_blocks) * 128 * 512,
    }


def _build(y_act=Y_ACT):
    sch = _schedule(y_act)
    a_groups, b_groups, order = sch["a_groups"], sch["b_groups"], sch["order"]
    n_wtiles = sch["n_wtiles"]
    na, nb = len(a_groups), len(b_groups)

    nc = bacc.Bacc("TRN2", debug=False, num_devices=NCORES)

    lhs_d = nc.dram_tensor("lhs", [5, NT * 128], F32, kind="ExternalInput")
    rhs_d = nc.dram_tensor("rhs", [5, N], F32, kind="ExternalInput")
    msk_d = nc.dram_tensor("msk", [128, 2048], F32, kind="ExternalInput")
    dve_d = nc.dram_tensor("dveacc", [128, na], F32, kind="ExternalOutput")
    act_d = nc.dram_tensor("actacc", [128, n_wtiles], F32, kind="ExternalOutput")
    actb_d = (
        nc.dram_tensor("actbacc", [128, nb], F32, kind="ExternalOutput")
        if nb
        else None
    )

    with tile.TileContext(nc) as tc, ExitStack() as ctx:
        const = ctx.enter_context(tc.tile_pool(name="const", bufs=1))
        psum = ctx.enter_context(tc.tile_pool(name="psum", bufs=2, space="PSUM"))
        wpool = ctx.enter_context(tc.tile_pool(name="w", bufs=2))
        dpool = ctx.enter_context(tc.tile_pool(name="d", bufs=2))
        tpool = ctx.enter_context(tc.tile_pool(name="t", bufs=2))
        spool = ctx.enter_context(tc.tile_pool(name="scr", bufs=1))
        s2pool = ctx.enter_context(tc.tile_pool(name="scr2", bufs=1))
        apool = ctx.enter_context(tc.tile_pool(name="acc", bufs=1))

        lhs = const.tile([5, NT * 128], F32)
        nc.sync.dma_start(lhs[:], lhs_d.ap())
        rhs = const.tile([5, N], F32)
        nc.sync.dma_start(rhs[:], rhs_d.ap())
        msk = const.tile([128, 2048], F32)
        nc.sync.dma_start(msk[:], msk_d.ap())

        dve_acc = apool.tile([128, na], F32, tag="dve_acc")
        act_acc = apool.tile([128, n_wtiles], F32, tag="act_acc")
        actb_acc = apool.tile([128, nb], F32, tag="actb_acc", name="actb_acc") if nb else None

        eps_bias = const.tile([128, 1], F32, tag="eps_bias")
        nc.gpsimd.memset(eps_bias[:], EPS_B)
        r_bias = const.tile([128, 1], F32, tag="r_bias")
        nc.gpsimd.memset(r_bias[:], float(R_F32))
        zeros = const.tile([128, 2048], F32, tag="zeros")
        nc.gpsimd.memset(zeros[:], 0.0)

        cur_w = None
        seg = 0
        wt = 0  # w-tile index
        ga_done = 0

        for kind, gi in order:
            grp = (a_groups if kind == "A" else b_groups)[gi]
            used = len(grp) * 512
            pt = psum.tile([128, 2048], F32)
            for idx, (t, c) in enumerate(grp):
                nc.tensor.matmul(
                    pt[:, idx * 512 : (idx + 1) * 512],
                    lhs[:, t * 128 : (t + 1) * 128],
                    rhs[:, c * 512 : (c + 1) * 512],
                    start=True,
                    stop=True,
                )
            if kind == "A":
                if cur_w is None:
                    cur_w = wpool.tile([128, 8192], F32)
                    seg = 0
                wseg = cur_w[:, seg * 2048 : seg * 2048 + used]
                is_diag = grp[0][0] == grp[0][1]
                # w = max(min(u, cap), mask); mask = cap on killed entries
                # (zeros elsewhere, which also clamps negative-rounded u).
                nc.vector.scalar_tensor_tensor(
                    out=wseg,
                    in0=pt[:, :used],
                    scalar=CAP,
                    in1=(msk if is_diag else zeros)[:, :used],
                    op0=ALU.min,
                    op1=ALU.max,
                    accum_out=dve_acc[:, gi : gi + 1],
                )
                seg += 1
                ga_done += 1
                if seg == 4 or ga_done == na:
                    ext = (seg - 1) * 2048 + used
                    scr = spool.tile([128, 8192], F32, tag="scr")
                    nc.scalar.activation(
                        scr[:, :ext],
                        cur_w[:, :ext],
                        AF.Sqrt,
                        accum_out=act_acc[:, wt : wt + 1],
                    )
                    wt += 1
                    cur_w = None
            else:
                dt_ = dpool.tile([128, 2048], F32)
                nc.scalar.activation(dt_[:, :used], pt[:, :used], AF.Sqrt, bias=eps_bias[:])
                tt = tpool.tile([128, 2048], F32)
                nc.scalar.activation(
                    tt[:, :used],
                    dt_[:, :used],
                    AF.Relu,
                    bias=r_bias[:],
                    scale=-1.0,
                )
                s2 = s2pool.tile([128, 2048], F32, tag="scr2")
                nc.scalar.activation(
                    s2[:, :used],
                    tt[:, :used],
                    AF.Square,
                    accum_out=actb_acc[:, gi : gi + 1],
                )

        nc.sync.dma_start(dve_d.ap(), dve_acc[:])
        nc.sync.dma_start(act_d.ap(), act_acc[:])
        if nb:
            nc.sync.dma_start(actb_d.ap(), actb_acc[:])

    nc.compile()
    return nc, sch


def _core_inputs(xyz, k):
    b, r0 = k // 4, k % 4
    rows = (
        np.arange(128)[None, :] + ((r0 + 4 * np.arange(NT))[:, None] * 128)
    ).reshape(-1)
    P = xyz[b][rows].astype(np.float32)  # (2048, 3)
    Q = xyz[b].astype(np.float32)  # (8192, 3)
    lhs = np.empty((5, NT * 128), np.float32)
    lhs[0:3] = P.T
    lhs[3] = (P * P).sum(-1)
    lhs[4] = 1.0
    rhs = np.empty((5, N), np.float32)
    rhs[0:3] = -2.0 * Q.T
    rhs[3] = 1.0
    rhs[4] = (Q * Q).sum(-1)
    base = np.where(
        np.arange(512)[None, :] <= r0 * 128 + np.arange(128)[:, None], CAP, 0.0
    ).astype(np.float32)
    msk = np.tile(base, (1, 4))
    return {"lhs": lhs, "rhs": rhs, "msk": np.ascontiguousarray(msk)}


LAST_RESULTS = None  # stashed BassKernelResults for test harness / profiling


def kernel(xyz):
    global LAST_RESULTS
    xyz = np.asarray(xyz, dtype=np.float32)
    assert xyz.shape == (B, N, D)
    if "prog" not in _CACHE:
        _CACHE["prog"] = _build()
    nc, sch = _CACHE["prog"]

    in_maps = [_core_inputs(xyz, k) for k in range(NCORES)]
    res = run_bass_kernel_spmd(nc, in_maps, core_ids=list(range(NCORES)))
    LAST_RESULTS = res

    r64 = float(R_F32)
    total = 0.0
    for k in range(NCORES):
        out = res.results[k]
        s_w = float(out["dveacc"].astype(np.float64).sum())
        s_sq = float(out["actacc"].astype(np.float64).sum())
        s_b = float(out["actbacc"].astype(np.float64).sum()) if "actbacc" in out else 0.0
        total += sch["n_a_entries"] * r64 * r64 + s_w - 2.0 * r64 * s_sq + s_b
    mean = 2.0 * total / (B * N * N)
    return np.array(mean, dtype=np.float32)


# revision 11
# speedup vs baseline: 1.0235x; 1.0235x over previous
"""Collision-regularizer loss kernel for 8x TRN2 NeuronCores (Bass/Tile).

Problem: xyz (2, 8192, 3) fp32 -> scalar loss
    loss = mean_{b,i,j} [i!=j] * relu(R - ||p_i - p_j||)^2,  R = 0.1

Algorithm per core (SPMD, identical program, different data):
  - Augmented Gram matmul (K=5): u_ij = |p_i|^2 + |p_j|^2 - 2 p_i.p_j
    computed by PE directly into PSUM, 128x512 per matmul.
  - Let w = min(max(u, 0), R^2). Then relu(R - sqrt(u))^2 = R^2 + w - 2R*sqrt(w).
    So per block we only need Sum(w) (fused accum on the DVE clamp op) and
    Sum(sqrt(w)) (fused accum on the ACT sqrt op). No separate reduction pass.
  - Only strictly-upper blocks of the NxN matrix are computed (host doubles
    the sum); the 16 diagonal-strip blocks use an additive mask input that
    raises killed entries (j <= i) to exactly R^2 => zero loss contribution.
  - Y_ACT blocks instead take a 3-op ACT-only path (sqrt, relu(R-d),
    square+accum) to balance ACT vs DVE load.

Sharding: core k handles batch k//4, row-blocks R = (k%4) + 4t for t=0..15.
Then R//4 == t, so every core runs the identical triangular schedule
(t, c) for c in t..15; the diagonal-mask offset (k%4)*128 is input data.
"""

import sys

import numpy as np

if "/opt/trn_rl_repo" not in sys.path:
    sys.path.insert(0, "/opt/trn_rl_repo")

from contextlib import ExitStack

import concourse.bacc as bacc
import concourse.mybir as mybir
import concourse.tile as tile
from concourse.bass_utils import run_bass_kernel_spmd

F32 = mybir.dt.float32
ALU = mybir.AluOpType
AF = mybir.ActivationFunctionType

B, N, D = 2, 8192, 3
NCORES = 8
NT = 16  # row-blocks (128 rows) per core
NCB = 16  # col-blocks (512 cols) per batch row
R_F32 = np.float32(0.1)
CAP = float(np.float32(R_F32 * R_F32))  # fp32(0.1)^2, the clamp cap
EPS_B = 4e-6  # sqrt guard on the ACT-only path (PE rounding can make u<0)
Y_ACT = 12  # blocks routed to the ACT-only path (load balance knob)

_CACHE = {}


def _schedule(y_act=Y_ACT):
    """Block lists and group structure, identical for all cores."""
    blocks = [(t, c) for t in range(NT) for c in range(t, NCB)]  # 136
    diag = [b for b in blocks if b[0] == b[1]]  # 16
    nondiag = [b for b in blocks if b[0] != b[1]]  # 120
    # ACT-only blocks: the ones farthest from the diagonal
    nd_sorted = sorted(nondiag, key=lambda b: (b[1] - b[0]), reverse=True)
    b_set = set(nd_sorted[:y_act])
    a_nondiag = [b for b in nondiag if b not in b_set]
    a_blocks = a_nondiag + diag  # diag groups last (mask DMA overlap)
    b_blocks = [b for b in nd_sorted[:y_act]]
    a_groups = [a_blocks[i : i + 4] for i in range(0, len(a_blocks), 4)]
    b_groups = [b_blocks[i : i + 4] for i in range(0, len(b_blocks), 4)]
    n_diag_groups = (len(diag) + 3) // 4
    # interleave B-groups evenly among the non-diag A-groups
    order = []  # list of ("A"|"B", group_index)
    na, nb = len(a_groups), len(b_groups)
    stride = max(1, (na - n_diag_groups) // (nb + 1)) if nb else na
    bi = 0
    for ai in range(na):
        order.append(("A", ai))
        if bi < nb and (ai + 1) % stride == 0 and ai < na - n_diag_groups:
            order.append(("B", bi))
            bi += 1
    while bi < nb:
        order.append(("B", bi))
        bi += 1
    n_wtiles = (len(a_groups) + 3) // 4
    return {
        "a_groups": a_groups,
        "b_groups": b_groups,
        "order": order,
        "n_wtiles": n_wtiles,
        "n_a_entries": len(a_blocks) * 128 * 512,
    }


def _build(y_act=Y_ACT):
    sch = _schedule(y_act)
    a_groups, b_groups, order = sch["a_groups"], sch["b_groups"], sch["order"]
    n_wtiles = sch["n_wtiles"]
    na, nb = len(a_groups), len(b_groups)

    nc = bacc.Bacc("TRN2", debug=False, num_devices=NCORES)

    lhs_d = nc.dram_tensor("lhs", [5, NT * 128], F32, kind="ExternalInput")
    rhs_d = nc.dram_tensor("rhs", [5, N], F32, kind="ExternalInput")
    msk_d = nc.dram_tensor("msk", [128, 2048], F32, kind="ExternalInput")
    dve_d = nc.dram_tensor("dveacc", [128, na], F32, kind="ExternalOutput")
    act_d = nc.dram_tensor("actacc", [128, na], F32, kind="ExternalOutput")
    actb_d = (
        nc.dram_tensor("actbacc", [128, nb], F32, kind="ExternalOutput")
        if nb
        else None
    )

    with tile.TileContext(nc) as tc, ExitStack() as ctx:
        const = ctx.enter_context(tc.tile_pool(name="const", bufs=1))
        psum = ctx.enter_context(tc.tile_pool(name="psum", bufs=2, space="PSUM"))
        wpool = ctx.enter_context(tc.tile_pool(name="w", bufs=4))
        dpool = ctx.enter_context(tc.tile_pool(name="d", bufs=2))
        tpool = ctx.enter_context(tc.tile_pool(name="t", bufs=2))
        spool = ctx.enter_context(tc.tile_pool(name="scr", bufs=2))
        s2pool = ctx.enter_context(tc.tile_pool(name="scr2", bufs=1))
        apool = ctx.enter_context(tc.tile_pool(name="acc", bufs=1))

        lhs = const.tile([5, NT * 128], F32)
        nc.sync.dma_start(lhs[:], lhs_d.ap())
        rhs = const.tile([5, N], F32)
        nc.sync.dma_start(rhs[:], rhs_d.ap())
        msk = const.tile([128, 2048], F32)
        nc.sync.dma_start(msk[:], msk_d.ap())

        dve_acc = apool.tile([128, na], F32, tag="dve_acc")
        act_acc = apool.tile([128, na], F32, tag="act_acc")
        actb_acc = apool.tile([128, nb], F32, tag="actb_acc", name="actb_acc") if nb else None

        eps_bias = const.tile([128, 1], F32, tag="eps_bias")
        nc.gpsimd.memset(eps_bias[:], EPS_B)
        r_bias = const.tile([128, 1], F32, tag="r_bias")
        nc.gpsimd.memset(r_bias[:], float(R_F32))
        zeros = const.tile([128, 2048], F32, tag="zeros")
        nc.gpsimd.memset(zeros[:], 0.0)

        for kind, gi in order:
            grp = (a_groups if kind == "A" else b_groups)[gi]
            used = len(grp) * 512
            pt = psum.tile([128, 2048], F32)
            for idx, (t, c) in enumerate(grp):
                nc.tensor.matmul(
                    pt[:, idx * 512 : (idx + 1) * 512],
                    lhs[:, t * 128 : (t + 1) * 128],
                    rhs[:, c * 512 : (c + 1) * 512],
                    start=True,
                    stop=True,
                )
            if kind == "A":
                wt_tile = wpool.tile([128, 2048], F32, tag="w", name="w")
                is_diag = grp[0][0] == grp[0][1]
                # w = max(min(u, cap), mask); mask = cap on killed entries
                # (zeros elsewhere, which also clamps negative-rounded u).
                nc.vector.scalar_tensor_tensor(
                    out=wt_tile[:, :used],
                    in0=pt[:, :used],
                    scalar=CAP,
                    in1=(msk if is_diag else zeros)[:, :used],
                    op0=ALU.min,
                    op1=ALU.max,
                    accum_out=dve_acc[:, gi : gi + 1],
                )
                scr = spool.tile([128, 2048], F32, tag="scr")
                nc.scalar.activation(
                    scr[:, :used],
                    wt_tile[:, :used],
                    AF.Sqrt,
                    accum_out=act_acc[:, gi : gi + 1],
                )
            else:
                dt_ = dpool.tile([128, 2048], F32)
                nc.scalar.activation(dt_[:, :used], pt[:, :used], AF.Sqrt, bias=eps_bias[:])
                tt = tpool.tile([128, 2048], F32)
                nc.scalar.activation(
                    tt[:, :used],
                    dt_[:, :used],
                    AF.Relu,
                    bias=r_bias[:],
                    scale=-1.0,
                )
                s2 = s2pool.tile([128, 2048], F32, tag="scr2")
                nc.scalar.activation(
                    s2[:, :used],
                    tt[:, :used],
                    AF.Square,
                    accum_out=actb_acc[:, gi : gi + 1],
                )

        nc.sync.dma_start(dve_d.ap(), dve_acc[:])
        nc.sync.dma_start(act_d.ap(), act_acc[:])
        if nb:
            nc.sync.dma_start(actb_d.ap(), actb_acc[:])

    nc.compile()
    return nc, sch


def _core_inputs(xyz, k):
    b, r0 = k // 4, k % 4
    rows = (
        np.arange(128)[None, :] + ((r0 + 4 * np.arange(NT))[:, None] * 128)
    ).reshape(-1)
    P = xyz[b][rows].astype(np.float32)  # (2048, 3)
    Q = xyz[b].astype(np.float32)  # (8192, 3)
    lhs = np.empty((5, NT * 128), np.float32)
    lhs[0:3] = P.T
    lhs[3] = (P * P).sum(-1)
    lhs[4] = 1.0
    rhs = np.empty((5, N), np.float32)
    rhs[0:3] = -2.0 * Q.T
    rhs[3] = 1.0
    rhs[4] = (Q * Q).sum(-1)
    base = np.where(
        np.arange(512)[None, :] <= r0 * 128 + np.arange(128)[:, None], CAP, 0.0
    ).astype(np.float32)
    msk = np.tile(base, (1, 4))
    return {"lhs": lhs, "rhs": rhs, "msk": np.ascontiguousarray(msk)}


LAST_RESULTS = None  # stashed BassKernelResults for test harness / profiling


def kernel(xyz):
    global LAST_RESULTS
    xyz = np.asarray(xyz, dtype=np.float32)
    assert xyz.shape == (B, N, D)
    if "prog" not in _CACHE:
        _CACHE["prog"] = _build()
    nc, sch = _CACHE["prog"]

    in_maps = [_core_inputs(xyz, k) for k in range(NCORES)]
    res = run_bass_kernel_spmd(nc, in_maps, core_ids=list(range(NCORES)))
    LAST_RESULTS = res

    r64 = float(R_F32)
    total = 0.0
    for k in range(NCORES):
        out = res.results[k]
        s_w = float(out["dveacc"].astype(np.float64).sum())
        s_sq = float(out["actacc"].astype(np.float64).sum())
        s_b = float(out["actbacc"].astype(np.float64).sum()) if "actbacc" in out else 0.0
        total += sch["n_a_entries"] * r64 * r64 + s_w - 2.0 * r64 * s_sq + s_b
    mean = 2.0 * total / (B * N * N)
    return np.array(mean, dtype=np.float32)


# revision 15
# speedup vs baseline: 2.4473x; 2.3910x over previous
"""Collision-regularizer loss kernel for 8x TRN2 NeuronCores (Bass/Tile).

Problem: xyz (2, 8192, 3) fp32 -> scalar loss
    loss = mean_{b,i,j} [i!=j] * relu(R - ||p_i - p_j||)^2,  R = 0.1

Algorithm per core (SPMD, identical program, different data):
  - Augmented Gram matmul (K=5): u_ij = |p_i|^2 + |p_j|^2 - 2 p_i.p_j
    computed by PE directly into PSUM, 128x512 per matmul.
  - Let w = min(max(u, 0), R^2). Then relu(R - sqrt(u))^2 = R^2 + w - 2R*sqrt(w).
    So per block we only need Sum(w) (fused accum on the DVE clamp op) and
    Sum(sqrt(w)) (fused accum on the ACT sqrt op). No separate reduction pass.
  - Only strictly-upper blocks of the NxN matrix are computed (host doubles
    the sum); the 16 diagonal-strip blocks use an additive mask input that
    raises killed entries (j <= i) to exactly R^2 => zero loss contribution.
  - Y_ACT blocks instead take a 3-op ACT-only path (sqrt, relu(R-d),
    square+accum) to balance ACT vs DVE load.

Sharding: core k handles batch k//4, row-blocks R = (k%4) + 4t for t=0..15.
Then R//4 == t, so every core runs the identical triangular schedule
(t, c) for c in t..15; the diagonal-mask offset (k%4)*128 is input data.
"""

import sys

import numpy as np

if "/opt/trn_rl_repo" not in sys.path:
    sys.path.insert(0, "/opt/trn_rl_repo")

from contextlib import ExitStack

import ml_dtypes
import concourse.bacc as bacc
import concourse.mybir as mybir
import concourse.tile as tile
from concourse.bass_utils import run_bass_kernel_spmd

BF16NP = ml_dtypes.bfloat16
F32 = mybir.dt.float32
BF16 = mybir.dt.bfloat16
KAUG = 24  # split-bf16 augmented contraction depth
ALU = mybir.AluOpType
AF = mybir.ActivationFunctionType

B, N, D = 2, 8192, 3
NCORES = 8
NT = 16  # row-blocks (128 rows) per core
NCB = 16  # col-blocks (512 cols) per batch row
R_F32 = np.float32(0.1)
CAP = float(np.float32(R_F32 * R_F32))  # fp32(0.1)^2, the clamp cap
EPS_B = 4e-6  # sqrt guard on the ACT-only path (PE rounding can make u<0)
Y_ACT = 12  # blocks routed to the ACT-only path (load balance knob)

_CACHE = {}


def _schedule(y_act=Y_ACT):
    """Block lists and group structure, identical for all cores."""
    blocks = [(t, c) for t in range(NT) for c in range(t, NCB)]  # 136
    diag = [b for b in blocks if b[0] == b[1]]  # 16
    nondiag = [b for b in blocks if b[0] != b[1]]  # 120
    # ACT-only blocks: the ones farthest from the diagonal
    nd_sorted = sorted(nondiag, key=lambda b: (b[1] - b[0]), reverse=True)
    b_set = set(nd_sorted[:y_act])
    a_nondiag = [b for b in nondiag if b not in b_set]
    a_blocks = a_nondiag + diag  # diag groups last (mask DMA overlap)
    b_blocks = [b for b in nd_sorted[:y_act]]
    a_groups = [a_blocks[i : i + 4] for i in range(0, len(a_blocks), 4)]
    b_groups = [b_blocks[i : i + 4] for i in range(0, len(b_blocks), 4)]
    n_diag_groups = (len(diag) + 3) // 4
    # interleave B-groups evenly among the non-diag A-groups
    order = []  # list of ("A"|"B", group_index)
    na, nb = len(a_groups), len(b_groups)
    stride = max(1, (na - n_diag_groups) // (nb + 1)) if nb else na
    bi = 0
    for ai in range(na):
        order.append(("A", ai))
        if bi < nb and (ai + 1) % stride == 0 and ai < na - n_diag_groups:
            order.append(("B", bi))
            bi += 1
    while bi < nb:
        order.append(("B", bi))
        bi += 1
    n_wtiles = (len(a_groups) + 3) // 4
    return {
        "a_groups": a_groups,
        "b_groups": b_groups,
        "order": order,
        "n_wtiles": n_wtiles,
        "n_a_entries": len(a_blocks) * 128 * 512,
    }


def _build(y_act=Y_ACT):
    sch = _schedule(y_act)
    a_groups, b_groups, order = sch["a_groups"], sch["b_groups"], sch["order"]
    n_wtiles = sch["n_wtiles"]
    na, nb = len(a_groups), len(b_groups)

    nc = bacc.Bacc("TRN2", debug=False, num_devices=NCORES)

    lhs_d = nc.dram_tensor("lhs", [KAUG, NT * 128], BF16, kind="ExternalInput")
    rhs_d = nc.dram_tensor("rhs", [KAUG, N], BF16, kind="ExternalInput")
    msk_d = nc.dram_tensor("msk", [128, 2048], F32, kind="ExternalInput")
    dve_d = nc.dram_tensor("dveacc", [128, na], F32, kind="ExternalOutput")
    act_d = nc.dram_tensor("actacc", [128, na], F32, kind="ExternalOutput")
    actb_d = (
        nc.dram_tensor("actbacc", [128, nb], F32, kind="ExternalOutput")
        if nb
        else None
    )

    with tile.TileContext(nc) as tc, ExitStack() as ctx:
        const = ctx.enter_context(tc.tile_pool(name="const", bufs=1))
        psum = ctx.enter_context(tc.tile_pool(name="psum", bufs=2, space="PSUM"))
        wpool = ctx.enter_context(tc.tile_pool(name="w", bufs=4))
        dpool = ctx.enter_context(tc.tile_pool(name="d", bufs=2))
        tpool = ctx.enter_context(tc.tile_pool(name="t", bufs=2))
        spool = ctx.enter_context(tc.tile_pool(name="scr", bufs=2))
        s2pool = ctx.enter_context(tc.tile_pool(name="scr2", bufs=1))
        apool = ctx.enter_context(tc.tile_pool(name="acc", bufs=1))

        lhs = const.tile([KAUG, NT * 128], BF16)
        nc.sync.dma_start(lhs[:], lhs_d.ap())
        rhs = const.tile([KAUG, N], BF16)
        nc.sync.dma_start(rhs[:], rhs_d.ap())
        msk = const.tile([128, 2048], F32)
        nc.sync.dma_start(msk[:], msk_d.ap())

        dve_acc = apool.tile([128, na], F32, tag="dve_acc")
        act_acc = apool.tile([128, na], F32, tag="act_acc")
        actb_acc = apool.tile([128, nb], F32, tag="actb_acc", name="actb_acc") if nb else None

        eps_bias = const.tile([128, 1], F32, tag="eps_bias")
        nc.gpsimd.memset(eps_bias[:], EPS_B)
        r_bias = const.tile([128, 1], F32, tag="r_bias")
        nc.gpsimd.memset(r_bias[:], float(R_F32))
        zeros = const.tile([128, 2048], F32, tag="zeros")
        nc.gpsimd.memset(zeros[:], 0.0)

        for kind, gi in order:
            grp = (a_groups if kind == "A" else b_groups)[gi]
            used = len(grp) * 512
            pt = psum.tile([128, 2048], F32)
            for idx, (t, c) in enumerate(grp):
                nc.tensor.matmul(
                    pt[:, idx * 512 : (idx + 1) * 512],
                    lhs[:, t * 128 : (t + 1) * 128],
                    rhs[:, c * 512 : (c + 1) * 512],
                    start=True,
                    stop=True,
                )
            if kind == "A":
                wt_tile = wpool.tile([128, 2048], F32, tag="w", name="w")
                is_diag = grp[0][0] == grp[0][1]
                # w = max(min(u, cap), mask); mask = cap on killed entries
                # (zeros elsewhere, which also clamps negative-rounded u).
                nc.vector.scalar_tensor_tensor(
                    out=wt_tile[:, :used],
                    in0=pt[:, :used],
                    scalar=CAP,
                    in1=(msk if is_diag else zeros)[:, :used],
                    op0=ALU.min,
                    op1=ALU.max,
                    accum_out=dve_acc[:, gi : gi + 1],
                )
                scr = spool.tile([128, 2048], F32, tag="scr")
                nc.scalar.activation(
                    scr[:, :used],
                    wt_tile[:, :used],
                    AF.Sqrt,
                    accum_out=act_acc[:, gi : gi + 1],
                )
            else:
                dt_ = dpool.tile([128, 2048], F32)
                nc.scalar.activation(dt_[:, :used], pt[:, :used], AF.Sqrt, bias=eps_bias[:])
                tt = tpool.tile([128, 2048], F32)
                nc.scalar.activation(
                    tt[:, :used],
                    dt_[:, :used],
                    AF.Relu,
                    bias=r_bias[:],
                    scale=-1.0,
                )
                s2 = s2pool.tile([128, 2048], F32, tag="scr2")
                nc.scalar.activation(
                    s2[:, :used],
                    tt[:, :used],
                    AF.Square,
                    accum_out=actb_acc[:, gi : gi + 1],
                )

        nc.sync.dma_start(dve_d.ap(), dve_acc[:])
        nc.sync.dma_start(act_d.ap(), act_acc[:])
        if nb:
            nc.sync.dma_start(actb_d.ap(), actb_acc[:])

    nc.compile()
    return nc, sch


def _split3(v):
    """3-level bf16 split: v ~= h + l + l2 (each exactly bf16-representable)."""
    h = v.astype(BF16NP).astype(np.float32)
    r = v - h
    l = r.astype(BF16NP).astype(np.float32)
    l2 = (r - l).astype(BF16NP).astype(np.float32)
    return h, l, l2


def _aug_rows(P):
    """Per-point split pieces. P: (n, 3) f32 -> dict of (n,) f32 arrays."""
    out = {}
    for i, cname in enumerate("xyz"):
        h, l, l2 = _split3(P[:, i])
        out[cname + "h"], out[cname + "l"], out[cname + "l2"] = h, l, l2
    s = (P * P).sum(-1)
    out["sh"], out["sl"], out["sl2"] = _split3(s)
    out["one"] = np.ones(len(P), np.float32)
    return out


# (lhs_piece, rhs_piece) per contraction row; rhs coord pieces get a -2 factor.
_PAIRS = (
    [("xh", "xh"), ("yh", "yh"), ("zh", "zh"), ("sh", "one"), ("one", "sh")]
    + [
        (c + a, c + b)
        for c in "xyz"
        for a, b in [("h", "l"), ("l", "h"), ("l", "l"), ("h", "l2"), ("l2", "h")]
    ]
    + [("sl", "one"), ("sl2", "one"), ("one", "sl"), ("one", "sl2")]
)
assert len(_PAIRS) == KAUG
_COORD_PIECES = {c + s for c in "xyz" for s in ("h", "l", "l2")}


def _core_inputs(xyz, k):
    b, r0 = k // 4, k % 4
    rows = (
        np.arange(128)[None, :] + ((r0 + 4 * np.arange(NT))[:, None] * 128)
    ).reshape(-1)
    P = xyz[b][rows].astype(np.float32)  # (2048, 3)
    Q = xyz[b].astype(np.float32)  # (8192, 3)
    pl, pr = _aug_rows(P), _aug_rows(Q)
    lhs = np.empty((KAUG, NT * 128), np.float32)
    rhs = np.empty((KAUG, N), np.float32)
    for i, (a, bname) in enumerate(_PAIRS):
        lhs[i] = pl[a]
        rhs[i] = (-2.0 * pr[bname]) if bname in _COORD_PIECES else pr[bname]
    base = np.where(
        np.arange(512)[None, :] <= r0 * 128 + np.arange(128)[:, None], CAP, 0.0
    ).astype(np.float32)
    msk = np.tile(base, (1, 4))
    return {
        "lhs": lhs.astype(BF16NP),
        "rhs": rhs.astype(BF16NP),
        "msk": np.ascontiguousarray(msk),
    }


LAST_RESULTS = None  # stashed BassKernelResults for test harness / profiling


def kernel(xyz):
    global LAST_RESULTS
    xyz = np.asarray(xyz, dtype=np.float32)
    assert xyz.shape == (B, N, D)
    if "prog" not in _CACHE:
        _CACHE["prog"] = _build()
    nc, sch = _CACHE["prog"]

    in_maps = [_core_inputs(xyz, k) for k in range(NCORES)]
    res = run_bass_kernel_spmd(nc, in_maps, core_ids=list(range(NCORES)))
    LAST_RESULTS = res

    r64 = float(R_F32)
    total = 0.0
    for k in range(NCORES):
        out = res.results[k]
        s_w = float(out["dveacc"].astype(np.float64).sum())
        s_sq = float(out["actacc"].astype(np.float64).sum())
        s_b = float(out["actbacc"].astype(np.float64).sum()) if "actbacc" in out else 0.0
        total += sch["n_a_entries"] * r64 * r64 + s_w - 2.0 * r64 * s_sq + s_b
    mean = 2.0 * total / (B * N * N)
    return np.array(mean, dtype=np.float32)


# revision 16
# speedup vs baseline: 2.7318x; 1.1163x over previous
"""Collision-regularizer loss kernel for 8x TRN2 NeuronCores (Bass/Tile).

Problem: xyz (2, 8192, 3) fp32 -> scalar loss
    loss = mean_{b,i,j} [i!=j] * relu(R - ||p_i - p_j||)^2,  R = 0.1

Algorithm per core (SPMD, identical program, different data):
  - Augmented Gram matmul (K=5): u_ij = |p_i|^2 + |p_j|^2 - 2 p_i.p_j
    computed by PE directly into PSUM, 128x512 per matmul.
  - Let w = min(max(u, 0), R^2). Then relu(R - sqrt(u))^2 = R^2 + w - 2R*sqrt(w).
    So per block we only need Sum(w) (fused accum on the DVE clamp op) and
    Sum(sqrt(w)) (fused accum on the ACT sqrt op). No separate reduction pass.
  - Only strictly-upper blocks of the NxN matrix are computed (host doubles
    the sum); the 16 diagonal-strip blocks use an additive mask input that
    raises killed entries (j <= i) to exactly R^2 => zero loss contribution.
  - Y_ACT blocks instead take a 3-op ACT-only path (sqrt, relu(R-d),
    square+accum) to balance ACT vs DVE load.

Sharding: core k handles batch k//4, row-blocks R = (k%4) + 4t for t=0..15.
Then R//4 == t, so every core runs the identical triangular schedule
(t, c) for c in t..15; the diagonal-mask offset (k%4)*128 is input data.
"""

import sys

import numpy as np

if "/opt/trn_rl_repo" not in sys.path:
    sys.path.insert(0, "/opt/trn_rl_repo")

from contextlib import ExitStack

import ml_dtypes
import concourse.bacc as bacc
import concourse.mybir as mybir
import concourse.tile as tile
from concourse.bass_utils import run_bass_kernel_spmd

BF16NP = ml_dtypes.bfloat16
F32 = mybir.dt.float32
BF16 = mybir.dt.bfloat16
KAUG = 24  # split-bf16 augmented contraction depth
ALU = mybir.AluOpType
AF = mybir.ActivationFunctionType

B, N, D = 2, 8192, 3
NCORES = 8
NT = 16  # row-blocks (128 rows) per core
NCB = 16  # col-blocks (512 cols) per batch row
R_F32 = np.float32(0.1)
CAP = float(np.float32(R_F32 * R_F32))  # fp32(0.1)^2, the clamp cap
EPS_B = 4e-6  # sqrt guard on the ACT-only path (PE rounding can make u<0)
Y_ACT = 4  # blocks routed to the ACT-only path (load balance knob)

_CACHE = {}


def _schedule(y_act=Y_ACT):
    """Block lists and group structure, identical for all cores."""
    blocks = [(t, c) for t in range(NT) for c in range(t, NCB)]  # 136
    diag = [b for b in blocks if b[0] == b[1]]  # 16
    nondiag = [b for b in blocks if b[0] != b[1]]  # 120
    # ACT-only blocks: the ones farthest from the diagonal
    nd_sorted = sorted(nondiag, key=lambda b: (b[1] - b[0]), reverse=True)
    b_set = set(nd_sorted[:y_act])
    a_nondiag = [b for b in nondiag if b not in b_set]
    a_blocks = a_nondiag + diag  # diag groups last (mask DMA overlap)
    b_blocks = [b for b in nd_sorted[:y_act]]
    a_groups = [a_blocks[i : i + 4] for i in range(0, len(a_blocks), 4)]
    b_groups = [b_blocks[i : i + 4] for i in range(0, len(b_blocks), 4)]
    n_diag_groups = (len(diag) + 3) // 4
    # interleave B-groups evenly among the non-diag A-groups
    order = []  # list of ("A"|"B", group_index)
    na, nb = len(a_groups), len(b_groups)
    stride = max(1, (na - n_diag_groups) // (nb + 1)) if nb else na
    bi = 0
    for ai in range(na):
        order.append(("A", ai))
        if bi < nb and (ai + 1) % stride == 0 and ai < na - n_diag_groups:
            order.append(("B", bi))
            bi += 1
    while bi < nb:
        order.append(("B", bi))
        bi += 1
    n_wtiles = (len(a_groups) + 3) // 4
    return {
        "a_groups": a_groups,
        "b_groups": b_groups,
        "order": order,
        "n_wtiles": n_wtiles,
        "n_a_entries": len(a_blocks) * 128 * 512,
    }


def _build(y_act=Y_ACT):
    sch = _schedule(y_act)
    a_groups, b_groups, order = sch["a_groups"], sch["b_groups"], sch["order"]
    n_wtiles = sch["n_wtiles"]
    na, nb = len(a_groups), len(b_groups)

    nc = bacc.Bacc("TRN2", debug=False, num_devices=NCORES)

    lhs_d = nc.dram_tensor("lhs", [KAUG, NT * 128], BF16, kind="ExternalInput")
    rhs_d = nc.dram_tensor("rhs", [KAUG, N], BF16, kind="ExternalInput")
    msk_d = nc.dram_tensor("msk", [128, 2048], F32, kind="ExternalInput")
    dve_d = nc.dram_tensor("dveacc", [128, na], F32, kind="ExternalOutput")
    act_d = nc.dram_tensor("actacc", [128, (na + 1) // 2], F32, kind="ExternalOutput")
    actb_d = (
        nc.dram_tensor("actbacc", [128, nb], F32, kind="ExternalOutput")
        if nb
        else None
    )

    with tile.TileContext(nc) as tc, ExitStack() as ctx:
        const = ctx.enter_context(tc.tile_pool(name="const", bufs=1))
        psum = ctx.enter_context(tc.tile_pool(name="psum", bufs=2, space="PSUM"))
        wpool = ctx.enter_context(tc.tile_pool(name="w", bufs=4))
        dpool = ctx.enter_context(tc.tile_pool(name="d", bufs=2))
        tpool = ctx.enter_context(tc.tile_pool(name="t", bufs=2))
        spool = ctx.enter_context(tc.tile_pool(name="scr", bufs=2))
        s2pool = ctx.enter_context(tc.tile_pool(name="scr2", bufs=1))
        apool = ctx.enter_context(tc.tile_pool(name="acc", bufs=1))

        lhs = const.tile([KAUG, NT * 128], BF16)
        nc.sync.dma_start(lhs[:], lhs_d.ap())
        rhs = const.tile([KAUG, N], BF16)
        for ch in range(4):
            nc.sync.dma_start(
                rhs[:, ch * 2048 : (ch + 1) * 2048],
                rhs_d.ap()[:, ch * 2048 : (ch + 1) * 2048],
            )
        msk = const.tile([128, 2048], F32)
        nc.sync.dma_start(msk[:], msk_d.ap())

        dve_acc = apool.tile([128, na], F32, tag="dve_acc")
        act_acc = apool.tile([128, (na + 1) // 2], F32, tag="act_acc")
        actb_acc = apool.tile([128, nb], F32, tag="actb_acc", name="actb_acc") if nb else None

        eps_bias = const.tile([128, 1], F32, tag="eps_bias")
        nc.gpsimd.memset(eps_bias[:], EPS_B)
        r_bias = const.tile([128, 1], F32, tag="r_bias")
        nc.gpsimd.memset(r_bias[:], float(R_F32))
        zeros = const.tile([128, 2048], F32, tag="zeros")
        nc.gpsimd.memset(zeros[:], 0.0)

        cur_w = None
        seg = 0
        wt = 0
        ga_done = 0

        for kind, gi in order:
            grp = (a_groups if kind == "A" else b_groups)[gi]
            used = len(grp) * 512
            pt = psum.tile([128, 2048], F32)
            for idx, (t, c) in enumerate(grp):
                nc.tensor.matmul(
                    pt[:, idx * 512 : (idx + 1) * 512],
                    lhs[:, t * 128 : (t + 1) * 128],
                    rhs[:, c * 512 : (c + 1) * 512],
                    start=True,
                    stop=True,
                )
            if kind == "A":
                if cur_w is None:
                    cur_w = wpool.tile([128, 4096], F32, tag="w", name="w")
                    seg = 0
                is_diag = grp[0][0] == grp[0][1]
                # w = max(min(u, cap), mask); mask = cap on killed entries
                # (zeros elsewhere, which also clamps negative-rounded u).
                nc.vector.scalar_tensor_tensor(
                    out=cur_w[:, seg * 2048 : seg * 2048 + used],
                    in0=pt[:, :used],
                    scalar=CAP,
                    in1=(msk if is_diag else zeros)[:, :used],
                    op0=ALU.min,
                    op1=ALU.max,
                    accum_out=dve_acc[:, gi : gi + 1],
                )
                seg += 1
                ga_done += 1
                if seg == 2 or ga_done == na:
                    ext = (seg - 1) * 2048 + used
                    scr = spool.tile([128, 4096], F32, tag="scr")
                    nc.scalar.activation(
                        scr[:, :ext],
                        cur_w[:, :ext],
                        AF.Sqrt,
                        accum_out=act_acc[:, wt : wt + 1],
                    )
                    wt += 1
                    cur_w = None
            else:
                dt_ = dpool.tile([128, 2048], F32)
                nc.scalar.activation(dt_[:, :used], pt[:, :used], AF.Sqrt, bias=eps_bias[:])
                tt = tpool.tile([128, 2048], F32)
                nc.scalar.activation(
                    tt[:, :used],
                    dt_[:, :used],
                    AF.Relu,
                    bias=r_bias[:],
                    scale=-1.0,
                )
                s2 = s2pool.tile([128, 2048], F32, tag="scr2")
                nc.scalar.activation(
                    s2[:, :used],
                    tt[:, :used],
                    AF.Square,
                    accum_out=actb_acc[:, gi : gi + 1],
                )

        nc.sync.dma_start(dve_d.ap(), dve_acc[:])
        nc.sync.dma_start(act_d.ap(), act_acc[:])
        if nb:
            nc.sync.dma_start(actb_d.ap(), actb_acc[:])

    nc.compile()
    return nc, sch


def _split3(v):
    """3-level bf16 split: v ~= h + l + l2 (each exactly bf16-representable)."""
    h = v.astype(BF16NP).astype(np.float32)
    r = v - h
    l = r.astype(BF16NP).astype(np.float32)
    l2 = (r - l).astype(BF16NP).astype(np.float32)
    return h, l, l2


def _aug_rows(P):
    """Per-point split pieces. P: (n, 3) f32 -> dict of (n,) f32 arrays."""
    out = {}
    for i, cname in enumerate("xyz"):
        h, l, l2 = _split3(P[:, i])
        out[cname + "h"], out[cname + "l"], out[cname + "l2"] = h, l, l2
    s = (P * P).sum(-1)
    out["sh"], out["sl"], out["sl2"] = _split3(s)
    out["one"] = np.ones(len(P), np.float32)
    return out


# (lhs_piece, rhs_piece) per contraction row; rhs coord pieces get a -2 factor.
_PAIRS = (
    [("xh", "xh"), ("yh", "yh"), ("zh", "zh"), ("sh", "one"), ("one", "sh")]
    + [
        (c + a, c + b)
        for c in "xyz"
        for a, b in [("h", "l"), ("l", "h"), ("l", "l"), ("h", "l2"), ("l2", "h")]
    ]
    + [("sl", "one"), ("sl2", "one"), ("one", "sl"), ("one", "sl2")]
)
assert len(_PAIRS) == KAUG
_COORD_PIECES = {c + s for c in "xyz" for s in ("h", "l", "l2")}


def _core_inputs(xyz, k):
    b, r0 = k // 4, k % 4
    rows = (
        np.arange(128)[None, :] + ((r0 + 4 * np.arange(NT))[:, None] * 128)
    ).reshape(-1)
    P = xyz[b][rows].astype(np.float32)  # (2048, 3)
    Q = xyz[b].astype(np.float32)  # (8192, 3)
    pl, pr = _aug_rows(P), _aug_rows(Q)
    lhs = np.empty((KAUG, NT * 128), np.float32)
    rhs = np.empty((KAUG, N), np.float32)
    for i, (a, bname) in enumerate(_PAIRS):
        lhs[i] = pl[a]
        rhs[i] = (-2.0 * pr[bname]) if bname in _COORD_PIECES else pr[bname]
    base = np.where(
        np.arange(512)[None, :] <= r0 * 128 + np.arange(128)[:, None], CAP, 0.0
    ).astype(np.float32)
    msk = np.tile(base, (1, 4))
    return {
        "lhs": lhs.astype(BF16NP),
        "rhs": rhs.astype(BF16NP),
        "msk": np.ascontiguousarray(msk),
    }


LAST_RESULTS = None  # stashed BassKernelResults for test harness / profiling


def kernel(xyz):
    global LAST_RESULTS
    xyz = np.asarray(xyz, dtype=np.float32)
    assert xyz.shape == (B, N, D)
    if "prog" not in _CACHE:
        _CACHE["prog"] = _build()
    nc, sch = _CACHE["prog"]

    in_maps = [_core_inputs(xyz, k) for k in range(NCORES)]
    res = run_bass_kernel_spmd(nc, in_maps, core_ids=list(range(NCORES)))
    LAST_RESULTS = res

    r64 = float(R_F32)
    total = 0.0
    for k in range(NCORES):
        out = res.results[k]
        s_w = float(out["dveacc"].astype(np.float64).sum())
        s_sq = float(out["actacc"].astype(np.float64).sum())
        s_b = float(out["actbacc"].astype(np.float64).sum()) if "actbacc" in out else 0.0
        total += sch["n_a_entries"] * r64 * r64 + s_w - 2.0 * r64 * s_sq + s_b
    mean = 2.0 * total / (B * N * N)
    return np.array(mean, dtype=np.float32)
